# revision 23
# baseline (speedup 1.0000x reference)
"""Causal ReLU-attention (no softmax) fused kernel for TRN2, 8 NeuronCores.

Reference computation (B=2, T=2048, C=1024, H=16, D=64):
    qkv = x @ W.T + b ; q,k,v split; per (b,h): y = relu(tril(q k^T / sqrt(D))) @ v

Sharding: core c handles batch b = c//4 and heads 4*(c%4) .. 4*(c%4)+3.
Each core is fully independent (no collectives).

Design (all matmuls bf16, fp32 PSUM accumulate; ~117us vs 155-166us
fp32r baseline):
  - Inputs host-converted to bf16: halves input DMA (5.9MB/core) and
    removes the fp32r <256-col matmul penalty.
  - Phase 1 (projection): per T-half, q/k matmuls interleaved per
    contraction chunk (tracks DMA arrival), 8 single-bank PSUM tiles;
    v bias folded into the accumulation as a rank-1 (ones x bv) matmul
    so every evac is a plain ACT/DVE copy writing bf16.  Input DMAs
    split across both HW queues (SP + ACT) in consumption order.
  - Phase 2 (attention): flat SW pipeline over all (qc, hp) tiles
    (order qc=1,2,0,3 so evac-heavy qc0 sits between PE-heavy windows;
    diag blocks spread between off-diag ones).  KEY perf fact: two
    matmuls with disjoint 64-row PE row-groups (the two heads' STs,
    64-dim contraction) OR disjoint 64-col col-groups (the two heads'
    AVs into one [128,512] yps bank) DUAL-ISSUE on the PE -- phase-2
    runs both heads' ST and AV pairs concurrently.  Do NOT interleave
    single ST/AV instructions; emit pairs back-to-back.
  - ReLU evacs rotate ACT/DVE (the only PSUM-capable engines; GPSIMD
    cannot touch PSUM).  Diag blocks alternate between a DVE fused
    scalar_tensor_tensor (relu + tril-mask multiply) and ACT relu +
    Pool in-SBUF mask multiply.  Phase 2 is evac-throughput-bound.
  - yps [128,512] PSUM holds both heads of a pair -> one copy + one
    contiguous output DMA per (hp,qc); double-buffered across its.
"""

import numpy as np

N_EMBD = 1024
N_HEAD = 16
HEAD_DIM = 64
B, T, C = 2, 2048, N_EMBD
NCORES = 8
P = 128
KC = C // P  # 8 contraction chunks
HT = T // 2  # 1024, half of T
NQC = T // 512  # 4 query chunks
NKB = T // P  # 16 key blocks

_NC_CACHE = {}


def _build_bass():
    import concourse.bass as bass
    from concourse import bacc, mybir
    from concourse.tile import TileContext

    f32 = mybir.dt.float32
    bf16 = mybir.dt.bfloat16

    nc = bacc.Bacc(None, target_bir_lowering=False)
    xt = nc.declare_dram_parameter("xt", [C, T], bf16, isOutput=False)
    wt = nc.declare_dram_parameter("wt", [C, 768], bf16, isOutput=False)
    bcol = nc.declare_dram_parameter("bcol", [512], f32, isOutput=False)
    bv = nc.declare_dram_parameter("bv", [256], bf16, isOutput=False)
    trim = nc.declare_dram_parameter("trim", [P, 512], bf16, isOutput=False)
    out = nc.declare_dram_parameter("out", [256, T], f32, isOutput=True)

    xt_r = xt[:, :].rearrange("(c p) t -> c p t", p=P)
    wt_r = wt[:, :].rearrange("(c p) o -> c p o", p=P)

    with TileContext(nc) as tc:
        with (
            tc.tile_pool(name="const", bufs=1) as const_pool,
            tc.tile_pool(name="qkv", bufs=1) as qkv_pool,
        ):
            # ---- input tiles (per-chunk so deps are fine-grained) ----
            wt_sb = [
                const_pool.tile([P, 768], bf16, name=f"wt{c}") for c in range(KC)
            ]
            xt_sb = [
                [
                    const_pool.tile([P, HT], bf16, name=f"xt{h}_{c}")
                    for c in range(KC)
                ]
                for h in range(2)
            ]
            bqk_sb = const_pool.tile([P, 4], f32)
            bv_sb = const_pool.tile([1, 256], bf16)
            ones_sb = const_pool.tile([1, P], bf16)
            trim_sb = const_pool.tile([P, 512], bf16)

            nc.vector.memset(ones_sb, 1.0)

            # DMA issue in consumption order, split across both HW queues:
            # wt chunks on the ACT queue, xt chunks on the SP queue, so the
            # first matmul's inputs stream concurrently.
            for c in range(KC):
                nc.scalar.dma_start(out=wt_sb[c], in_=wt_r[c])
                eng = nc.sync if c % 2 == 0 else nc.scalar
                eng.dma_start(out=xt_sb[0][c], in_=xt_r[c][:, 0:HT])
                if c == 1:
                    nc.sync.dma_start(
                        out=bqk_sb, in_=bcol[:].rearrange("(m p) -> p m", p=P)
                    )
                    nc.sync.dma_start(
                        out=bv_sb, in_=bv[:].rearrange("(a b) -> a b", a=1)
                    )
                    nc.sync.dma_start(out=trim_sb, in_=trim[:, :])
            for c in range(KC):
                eng = nc.sync if c % 2 == 0 else nc.scalar
                eng.dma_start(out=xt_sb[1][c], in_=xt_r[c][:, HT:T])

            q_sb = qkv_pool.tile([P, 2, T], bf16)
            k_sb = qkv_pool.tile([P, 2, T], bf16)
            v_sb = qkv_pool.tile([P, NKB, 256], bf16)

            # ---- phase 1: projection, by T-halves ----
            with tc.tile_pool(name="psum1", bufs=1, space="PSUM") as psum1:
                for half in range(2):
                    t0c = half * HT
                    pq = {}
                    pk = {}
                    for m in range(2):
                        for n in range(2):
                            pq[(m, n)] = psum1.tile(
                                [P, 512], f32, tag=f"p{2 * m + n}",
                                name=f"pq{half}{m}{n}",
                            )
                            pk[(m, n)] = psum1.tile(
                                [P, 512], f32, tag=f"p{4 + 2 * m + n}",
                                name=f"pk{half}{m}{n}",
                            )
                    for c in range(KC):
                        for m in range(2):
                            for n in range(2):
                                nc.tensor.matmul(
                                    pq[(m, n)],
                                    wt_sb[c][:, m * P : (m + 1) * P],
                                    xt_sb[half][c][:, n * 512 : (n + 1) * 512],
                                    start=(c == 0),
                                    stop=(c == KC - 1),
                                )
                        for m in range(2):
                            for n in range(2):
                                nc.tensor.matmul(
                                    pk[(m, n)],
                                    wt_sb[c][:, 256 + m * P : 256 + (m + 1) * P],
                                    xt_sb[half][c][:, n * 512 : (n + 1) * 512],
                                    start=(c == 0),
                                    stop=(c == KC - 1),
                                )
                    # evacs: q on ACT (bias via activation), k on DVE
                    for m in range(2):
                        for n in range(2):
                            sl = slice(t0c + n * 512, t0c + (n + 1) * 512)
                            nc.scalar.activation(
                                out=q_sb[:, m, sl],
                                in_=pq[(m, n)],
                                func=mybir.ActivationFunctionType.Identity,
                                bias=bqk_sb[:, m : m + 1],
                                scale=1.0,
                            )
                            nc.vector.tensor_scalar_add(
                                k_sb[:, m, sl], pk[(m, n)], bqk_sb[:, 2 + m : 3 + m]
                            )
                    # v: 4 psum tiles of [128,512], each covering 2 t-blocks;
                    # reuse q's tags (q evacs are issued first).  bv is folded
                    # in as a rank-1 (ones x bv) matmul so the evac is a copy.
                    for vt in range(4):
                        pv = psum1.tile(
                            [P, 512], f32, tag=f"p{vt}", name=f"pv{half}{vt}"
                        )
                        for tl in range(2):
                            tb8 = vt * 2 + tl
                            for c in range(KC):
                                nc.tensor.matmul(
                                    pv[:, tl * 256 : (tl + 1) * 256],
                                    xt_sb[half][c][:, tb8 * P : (tb8 + 1) * P],
                                    wt_sb[c][:, 512:768],
                                    start=(c == 0),
                                    stop=False,
                                )
                            nc.tensor.matmul(
                                pv[:, tl * 256 : (tl + 1) * 256],
                                ones_sb[0:1, :],
                                bv_sb[0:1, :],
                                start=False,
                                stop=True,
                            )
                        vdst = v_sb[:, half * 8 + vt * 2 : half * 8 + vt * 2 + 2, :]
                        pvr = pv.rearrange("p (a b) -> p a b", a=2)
                        if vt % 2 == 0:
                            nc.scalar.copy(vdst, pvr)
                        else:
                            nc.vector.tensor_copy(vdst, pvr)

            # ---- phase 2: attention, flat pipeline over (qc, hp) ----
            with (
                tc.tile_pool(name="stsb2", bufs=1) as stsb_pool,
                tc.tile_pool(name="ysb2", bufs=1) as ysb_pool,
                tc.tile_pool(name="pst", bufs=1, space="PSUM") as pst_pool,
                tc.tile_pool(name="py", bufs=1, space="PSUM") as py_pool,
            ):
                # iteration list: evac-heavy qc0 windows sandwiched between
                # PE-heavy qc2/qc3 windows; diag blocks spread between
                # off-diag ones so ST bursts never exceed the stps slots.
                its = []
                for qc in (1, 2, 0, 3):
                    for hp in range(2):
                        off = list(range(4 * qc))
                        diag = [4 * qc + dd for dd in range(4)]
                        kbs = []
                        for i4 in range(4):
                            kbs += off[i4 * qc : (i4 + 1) * qc] + [diag[i4]]
                        its.append((hp, qc, kbs))

                yps = {}  # it_idx -> psum tile (both heads, 1 bank)
                n_av = {}  # it_idx -> AV blocks emitted
                eload = [0.0, 0.0]  # ACT, DVE cumulative evac cost

                def emit_st(it_idx, hp, qc, kb):
                    d = kb - 4 * qc
                    c0 = P * d if d > 0 else 0
                    stps = pst_pool.tile(
                        [P, 2, 512], f32, tag="stps", bufs=3, name="stps"
                    )
                    stsb = stsb_pool.tile(
                        [P, 2, 512], bf16, tag="stsb", bufs=8, name="stsb"
                    )
                    for hh in range(2):
                        off = hh * 64
                        nc.tensor.matmul(
                            stps[:, hh, c0:512],
                            k_sb[off : off + 64, hp, kb * P : (kb + 1) * P],
                            q_sb[off : off + 64, hp, qc * 512 + c0 : (qc + 1) * 512],
                            start=True,
                            stop=True,
                        )
                    L = 512 - c0
                    act_light = eload[0] * 1.25 <= eload[1]
                    if d >= 0 and not act_light:
                        # diag: fused relu + causal-triangle mask on DVE
                        mask = bass.AP(
                            tensor=trim_sb.tensor,
                            offset=trim_sb.offset,
                            ap=[trim_sb.ap[0], [0, 2], [1, L]],
                        )
                        nc.vector.scalar_tensor_tensor(
                            out=stsb[:, :, c0:512],
                            in0=stps[:, :, c0:512],
                            scalar=0.0,
                            in1=mask,
                            op0=mybir.AluOpType.max,
                            op1=mybir.AluOpType.mult,
                        )
                        eload[1] += 2 * L * 1.04 + 170
                    else:
                        # plain relu on the less-loaded engine
                        if act_light:
                            nc.scalar.activation(
                                out=stsb[:, :, c0:512],
                                in_=stps[:, :, c0:512],
                                func=mybir.ActivationFunctionType.Relu,
                            )
                            eload[0] += 2 * L * 0.83 + 200
                        else:
                            nc.vector.tensor_scalar_max(
                                stsb[:, :, c0:512], stps[:, :, c0:512], 0.0
                            )
                            eload[1] += 2 * L * 1.04 + 170
                        if d >= 0:
                            # causal triangle applied in SBUF on the Pool
                            # engine (per-head 2D ops; Pool cannot read PSUM)
                            for hh in range(2):
                                nc.gpsimd.tensor_tensor(
                                    out=stsb[:, hh, c0 : c0 + P],
                                    in0=stsb[:, hh, c0 : c0 + P],
                                    in1=trim_sb[:, 0:P],
                                    op=mybir.AluOpType.mult,
                                )
                    return stsb

                def emit_av(it_idx, hp, qc, kb, stsb, nblocks):
                    d = kb - 4 * qc
                    c0 = P * d if d > 0 else 0
                    if it_idx not in yps:
                        yps[it_idx] = py_pool.tile(
                            [P, 512], f32, tag=f"yps{it_idx % 2}", name=f"yps{it_idx}"
                        )
                        n_av[it_idx] = 0
                    i = n_av[it_idx]
                    n_av[it_idx] = i + 1
                    for hh in range(2):
                        h = 2 * hp + hh
                        nc.tensor.matmul(
                            yps[it_idx][hh * 64 : (hh + 1) * 64, c0:512],
                            v_sb[:, kb, h * 64 : (h + 1) * 64],
                            stsb[:, hh, c0:512],
                            start=(i == 0),
                            stop=(i == nblocks - 1),
                        )
                    if i == nblocks - 1:
                        # drain yps -> sbuf -> dram (both heads at once)
                        ysb = ysb_pool.tile(
                            [P, 512], f32, tag="ysb", bufs=2, name="ysb"
                        )
                        if eload[0] * 1.25 <= eload[1]:
                            nc.scalar.copy(ysb, yps[it_idx])
                            eload[0] += 512 * 0.83 + 200
                        else:
                            nc.vector.tensor_copy(ysb, yps[it_idx])
                            eload[1] += 512 * 1.04 + 170
                        nc.sync.dma_start(
                            out=out[hp * P : (hp + 1) * P, qc * 512 : (qc + 1) * 512],
                            in_=ysb,
                        )

                SKEW = 4  # in blocks
                pending = []
                for it_idx, (hp, qc, kbs) in enumerate(its):
                    nblocks = len(kbs)
                    for kb in kbs:
                        stsb = emit_st(it_idx, hp, qc, kb)
                        pending.append((it_idx, hp, qc, kb, stsb, nblocks))
                        if len(pending) > SKEW:
                            emit_av(*pending.pop(0))
                for args in pending:
                    emit_av(*args)

    nc.compile()
    return nc


def _get_nc():
    if "nc" not in _NC_CACHE:
        _NC_CACHE["nc"] = _build_bass()
    return _NC_CACHE["nc"]


def make_in_maps(x: np.ndarray, W: np.ndarray, b: np.ndarray):
    import ml_dtypes

    bf = ml_dtypes.bfloat16
    scale = np.float32(1.0 / np.sqrt(HEAD_DIM))

    # tril-style mask: keep score[key p, query j] when j >= p; cols >= 128
    # (beyond the diagonal tile) are always kept.
    trim = np.ones((P, 512), dtype=bf)
    trim[:, 0:128] = (np.arange(128)[None, :] >= np.arange(P)[:, None]).astype(bf)
    trim = np.ascontiguousarray(trim)

    xts = [np.ascontiguousarray(x[bb].T.astype(bf)) for bb in range(B)]
    in_maps = []
    for core in range(NCORES):
        bb, g = core // 4, core % 4
        o0 = g * 256
        wq = W[o0 : o0 + 256, :] * scale
        wk = W[C + o0 : C + o0 + 256, :]
        wv = W[2 * C + o0 : 2 * C + o0 + 256, :]
        wt = np.ascontiguousarray(
            np.concatenate([wq.T, wk.T, wv.T], axis=1).astype(bf)
        )
        bq = b[o0 : o0 + 256] * scale
        bk = b[C + o0 : C + o0 + 256]
        bvv = np.ascontiguousarray(b[2 * C + o0 : 2 * C + o0 + 256].astype(bf))
        bcol = np.ascontiguousarray(np.concatenate([bq, bk]), dtype=np.float32)
        in_maps.append(
            {"xt": xts[bb], "wt": wt, "bcol": bcol, "bv": bvv, "trim": trim}
        )
    return in_maps


def kernel(x: np.ndarray, W: np.ndarray, b: np.ndarray) -> np.ndarray:
    from concourse.bass_utils import run_bass_kernel_spmd

    x = np.asarray(x, dtype=np.float32)
    W = np.asarray(W, dtype=np.float32)
    b = np.asarray(b, dtype=np.float32)

    nc = _get_nc()
    in_maps = make_in_maps(x, W, b)
    res = run_bass_kernel_spmd(nc, in_maps, core_ids=list(range(NCORES)))

    y = np.empty((B, T, C), dtype=np.float32)
    for core in range(NCORES):
        bb, g = core // 4, core % 4
        y[bb, :, g * 256 : (g + 1) * 256] = res.results[core]["out"].T
    return y


# revision 25
# speedup vs baseline: 1.0007x; 1.0007x over previous
"""Causal ReLU-attention (no softmax) fused kernel for TRN2, 8 NeuronCores.

Reference computation (B=2, T=2048, C=1024, H=16, D=64):
    qkv = x @ W.T + b ; q,k,v split; per (b,h): y = relu(tril(q k^T / sqrt(D))) @ v

Sharding: core c handles batch b = c//4 and heads 4*(c%4) .. 4*(c%4)+3.
Each core is fully independent (no collectives).

Design (all matmuls bf16, fp32 PSUM accumulate; ~117us vs 155-166us
fp32r baseline):
  - Inputs host-converted to bf16: halves input DMA (5.9MB/core) and
    removes the fp32r <256-col matmul penalty.
  - Phase 1 (projection): per T-half, q/k matmuls interleaved per
    contraction chunk (tracks DMA arrival), 8 single-bank PSUM tiles;
    v bias folded into the accumulation as a rank-1 (ones x bv) matmul
    so every evac is a plain ACT/DVE copy writing bf16.  Input DMAs
    split across both HW queues (SP + ACT) in consumption order.
  - Phase 2 (attention): flat SW pipeline over all (qc, hp) tiles
    (order qc=1,2,0,3 so evac-heavy qc0 sits between PE-heavy windows;
    diag blocks spread between off-diag ones).  KEY perf fact: two
    matmuls with disjoint 64-row PE row-groups (the two heads' STs,
    64-dim contraction) OR disjoint 64-col col-groups (the two heads'
    AVs into one [128,512] yps bank) DUAL-ISSUE on the PE -- phase-2
    runs both heads' ST and AV pairs concurrently.  Do NOT interleave
    single ST/AV instructions; emit pairs back-to-back.
  - ReLU evacs rotate ACT/DVE (the only PSUM-capable engines; GPSIMD
    cannot touch PSUM).  Diag blocks alternate between a DVE fused
    scalar_tensor_tensor (relu + tril-mask multiply) and ACT relu +
    Pool in-SBUF mask multiply.  Phase 2 is evac-throughput-bound.
  - yps [128,512] PSUM holds both heads of a pair -> one copy + one
    contiguous output DMA per (hp,qc); double-buffered across its.
"""

import numpy as np

N_EMBD = 1024
N_HEAD = 16
HEAD_DIM = 64
B, T, C = 2, 2048, N_EMBD
NCORES = 8
P = 128
KC = C // P  # 8 contraction chunks
HT = T // 2  # 1024, half of T
NQC = T // 512  # 4 query chunks
NKB = T // P  # 16 key blocks

_NC_CACHE = {}


def _build_bass():
    import concourse.bass as bass
    from concourse import bacc, mybir
    from concourse.tile import TileContext

    f32 = mybir.dt.float32
    bf16 = mybir.dt.bfloat16

    nc = bacc.Bacc(None, target_bir_lowering=False)
    xt = nc.declare_dram_parameter("xt", [C, T], bf16, isOutput=False)
    wt = nc.declare_dram_parameter("wt", [C, 768], bf16, isOutput=False)
    bcol = nc.declare_dram_parameter("bcol", [512], f32, isOutput=False)
    bv = nc.declare_dram_parameter("bv", [256], bf16, isOutput=False)
    trim = nc.declare_dram_parameter("trim", [P, 512], bf16, isOutput=False)
    out = nc.declare_dram_parameter("out", [256, T], f32, isOutput=True)

    xt_r = xt[:, :].rearrange("(c p) t -> c p t", p=P)
    wt_r = wt[:, :].rearrange("(c p) o -> c p o", p=P)

    with TileContext(nc) as tc:
        with (
            tc.tile_pool(name="const", bufs=1) as const_pool,
            tc.tile_pool(name="qkv", bufs=1) as qkv_pool,
        ):
            # ---- input tiles (per-chunk so deps are fine-grained) ----
            wt_sb = [
                const_pool.tile([P, 768], bf16, name=f"wt{c}") for c in range(KC)
            ]
            xt_sb = [
                [
                    const_pool.tile([P, HT], bf16, name=f"xt{h}_{c}")
                    for c in range(KC)
                ]
                for h in range(2)
            ]
            bqk_sb = const_pool.tile([P, 4], f32)
            bv_sb = const_pool.tile([1, 256], bf16)
            ones_sb = const_pool.tile([1, P], bf16)
            trim_sb = const_pool.tile([P, 512], bf16)
            warm_sb = const_pool.tile([P, 512], bf16)

            nc.vector.memset(ones_sb, 1.0)
            nc.vector.memset(warm_sb, 0.0)

            # DMA issue in consumption order, split across both HW queues:
            # wt chunks on the ACT queue, xt chunks on the SP queue, so the
            # first matmul's inputs stream concurrently.
            for c in range(KC):
                nc.scalar.dma_start(out=wt_sb[c], in_=wt_r[c])
                eng = nc.sync if c % 2 == 0 else nc.scalar
                eng.dma_start(out=xt_sb[0][c], in_=xt_r[c][:, 0:HT])
                if c == 1:
                    nc.sync.dma_start(
                        out=bqk_sb, in_=bcol[:].rearrange("(m p) -> p m", p=P)
                    )
                    nc.sync.dma_start(
                        out=bv_sb, in_=bv[:].rearrange("(a b) -> a b", a=1)
                    )
                    nc.sync.dma_start(out=trim_sb, in_=trim[:, :])
            for c in range(KC):
                eng = nc.sync if c % 2 == 0 else nc.scalar
                eng.dma_start(out=xt_sb[1][c], in_=xt_r[c][:, HT:T])

            q_sb = qkv_pool.tile([P, 2, T], bf16)
            k_sb = qkv_pool.tile([P, 2, T], bf16)
            v_sb = qkv_pool.tile([P, NKB, 256], bf16)

            # ---- phase 1: projection, by T-halves ----
            with tc.tile_pool(name="psum1", bufs=1, space="PSUM") as psum1:
                # PE warm-up: full-array dummy matmuls during the DMA lead-in
                # so the DVFS clock is ramped when the first real chunk lands.
                warmps = psum1.tile([P, 512], f32, tag="p7", name="warmps")
                for w in range(10):
                    nc.tensor.matmul(
                        warmps,
                        warm_sb[:, 0:P],
                        warm_sb[:, :],
                        start=(w == 0),
                        stop=(w == 9),
                    )
                for half in range(2):
                    t0c = half * HT
                    pq = {}
                    pk = {}
                    for m in range(2):
                        for n in range(2):
                            pq[(m, n)] = psum1.tile(
                                [P, 512], f32, tag=f"p{2 * m + n}",
                                name=f"pq{half}{m}{n}",
                            )
                            pk[(m, n)] = psum1.tile(
                                [P, 512], f32, tag=f"p{4 + 2 * m + n}",
                                name=f"pk{half}{m}{n}",
                            )
                    for c in range(KC):
                        for m in range(2):
                            for n in range(2):
                                nc.tensor.matmul(
                                    pq[(m, n)],
                                    wt_sb[c][:, m * P : (m + 1) * P],
                                    xt_sb[half][c][:, n * 512 : (n + 1) * 512],
                                    start=(c == 0),
                                    stop=(c == KC - 1),
                                )
                        for m in range(2):
                            for n in range(2):
                                nc.tensor.matmul(
                                    pk[(m, n)],
                                    wt_sb[c][:, 256 + m * P : 256 + (m + 1) * P],
                                    xt_sb[half][c][:, n * 512 : (n + 1) * 512],
                                    start=(c == 0),
                                    stop=(c == KC - 1),
                                )
                    # evacs: q on ACT (bias via activation), k on DVE
                    for m in range(2):
                        for n in range(2):
                            sl = slice(t0c + n * 512, t0c + (n + 1) * 512)
                            nc.scalar.activation(
                                out=q_sb[:, m, sl],
                                in_=pq[(m, n)],
                                func=mybir.ActivationFunctionType.Identity,
                                bias=bqk_sb[:, m : m + 1],
                                scale=1.0,
                            )
                            nc.vector.tensor_scalar_add(
                                k_sb[:, m, sl], pk[(m, n)], bqk_sb[:, 2 + m : 3 + m]
                            )
                    # v: 4 psum tiles of [128,512], each covering 2 t-blocks;
                    # reuse q's tags (q evacs are issued first).  bv is folded
                    # in as a rank-1 (ones x bv) matmul so the evac is a copy.
                    for vt in range(4):
                        pv = psum1.tile(
                            [P, 512], f32, tag=f"p{vt}", name=f"pv{half}{vt}"
                        )
                        for tl in range(2):
                            tb8 = vt * 2 + tl
                            for c in range(KC):
                                nc.tensor.matmul(
                                    pv[:, tl * 256 : (tl + 1) * 256],
                                    xt_sb[half][c][:, tb8 * P : (tb8 + 1) * P],
                                    wt_sb[c][:, 512:768],
                                    start=(c == 0),
                                    stop=False,
                                )
                            nc.tensor.matmul(
                                pv[:, tl * 256 : (tl + 1) * 256],
                                ones_sb[0:1, :],
                                bv_sb[0:1, :],
                                start=False,
                                stop=True,
                            )
                        vdst = v_sb[:, half * 8 + vt * 2 : half * 8 + vt * 2 + 2, :]
                        pvr = pv.rearrange("p (a b) -> p a b", a=2)
                        if vt % 2 == 0:
                            nc.scalar.copy(vdst, pvr)
                        else:
                            nc.vector.tensor_copy(vdst, pvr)

            # ---- phase 2: attention, flat pipeline over (qc, hp) ----
            with (
                tc.tile_pool(name="stsb2", bufs=1) as stsb_pool,
                tc.tile_pool(name="ysb2", bufs=1) as ysb_pool,
                tc.tile_pool(name="pst", bufs=1, space="PSUM") as pst_pool,
                tc.tile_pool(name="py", bufs=1, space="PSUM") as py_pool,
            ):
                # iteration list: evac-heavy qc0 windows sandwiched between
                # PE-heavy qc2/qc3 windows; diag blocks spread between
                # off-diag ones so ST bursts never exceed the stps slots.
                its = []
                for qc in (1, 2, 0, 3):
                    for hp in range(2):
                        off = list(range(4 * qc))
                        diag = [4 * qc + dd for dd in range(4)]
                        kbs = []
                        for i4 in range(4):
                            kbs += off[i4 * qc : (i4 + 1) * qc] + [diag[i4]]
                        its.append((hp, qc, kbs))

                yps = {}  # it_idx -> psum tile (both heads, 1 bank)
                n_av = {}  # it_idx -> AV blocks emitted
                eload = [0.0, 0.0]  # ACT, DVE cumulative evac cost

                def emit_st(it_idx, hp, qc, kb):
                    d = kb - 4 * qc
                    c0 = P * d if d > 0 else 0
                    stps = pst_pool.tile(
                        [P, 2, 512], f32, tag="stps", bufs=3, name="stps"
                    )
                    stsb = stsb_pool.tile(
                        [P, 2, 512], bf16, tag="stsb", bufs=8, name="stsb"
                    )
                    for hh in range(2):
                        off = hh * 64
                        nc.tensor.matmul(
                            stps[:, hh, c0:512],
                            k_sb[off : off + 64, hp, kb * P : (kb + 1) * P],
                            q_sb[off : off + 64, hp, qc * 512 + c0 : (qc + 1) * 512],
                            start=True,
                            stop=True,
                        )
                    L = 512 - c0
                    act_light = eload[0] * 1.25 <= eload[1]
                    if d >= 0 and not act_light:
                        # diag: fused relu + causal-triangle mask on DVE
                        mask = bass.AP(
                            tensor=trim_sb.tensor,
                            offset=trim_sb.offset,
                            ap=[trim_sb.ap[0], [0, 2], [1, L]],
                        )
                        nc.vector.scalar_tensor_tensor(
                            out=stsb[:, :, c0:512],
                            in0=stps[:, :, c0:512],
                            scalar=0.0,
                            in1=mask,
                            op0=mybir.AluOpType.max,
                            op1=mybir.AluOpType.mult,
                        )
                        eload[1] += 2 * L * 1.04 + 170
                    else:
                        # plain relu on the less-loaded engine
                        if act_light:
                            nc.scalar.activation(
                                out=stsb[:, :, c0:512],
                                in_=stps[:, :, c0:512],
                                func=mybir.ActivationFunctionType.Relu,
                            )
                            eload[0] += 2 * L * 0.83 + 200
                        else:
                            nc.vector.tensor_scalar_max(
                                stsb[:, :, c0:512], stps[:, :, c0:512], 0.0
                            )
                            eload[1] += 2 * L * 1.04 + 170
                        if d >= 0:
                            # causal triangle applied in SBUF on the Pool
                            # engine (per-head 2D ops; Pool cannot read PSUM)
                            for hh in range(2):
                                nc.gpsimd.tensor_tensor(
                                    out=stsb[:, hh, c0 : c0 + P],
                                    in0=stsb[:, hh, c0 : c0 + P],
                                    in1=trim_sb[:, 0:P],
                                    op=mybir.AluOpType.mult,
                                )
                    return stsb

                def emit_av(it_idx, hp, qc, kb, stsb, nblocks):
                    d = kb - 4 * qc
                    c0 = P * d if d > 0 else 0
                    if it_idx not in yps:
                        yps[it_idx] = py_pool.tile(
                            [P, 512], f32, tag=f"yps{it_idx % 2}", name=f"yps{it_idx}"
                        )
                        n_av[it_idx] = 0
                    i = n_av[it_idx]
                    n_av[it_idx] = i + 1
                    for hh in range(2):
                        h = 2 * hp + hh
                        nc.tensor.matmul(
                            yps[it_idx][hh * 64 : (hh + 1) * 64, c0:512],
                            v_sb[:, kb, h * 64 : (h + 1) * 64],
                            stsb[:, hh, c0:512],
                            start=(i == 0),
                            stop=(i == nblocks - 1),
                        )
                    if i == nblocks - 1:
                        # drain yps -> sbuf -> dram (both heads at once)
                        ysb = ysb_pool.tile(
                            [P, 512], f32, tag="ysb", bufs=2, name="ysb"
                        )
                        if eload[0] * 1.25 <= eload[1]:
                            nc.scalar.copy(ysb, yps[it_idx])
                            eload[0] += 512 * 0.83 + 200
                        else:
                            nc.vector.tensor_copy(ysb, yps[it_idx])
                            eload[1] += 512 * 1.04 + 170
                        nc.sync.dma_start(
                            out=out[hp * P : (hp + 1) * P, qc * 512 : (qc + 1) * 512],
                            in_=ysb,
                        )

                SKEW = 4  # in blocks
                pending = []
                for it_idx, (hp, qc, kbs) in enumerate(its):
                    nblocks = len(kbs)
                    for kb in kbs:
                        stsb = emit_st(it_idx, hp, qc, kb)
                        pending.append((it_idx, hp, qc, kb, stsb, nblocks))
                        if len(pending) > SKEW:
                            emit_av(*pending.pop(0))
                for args in pending:
                    emit_av(*args)

    nc.compile()
    return nc


def _get_nc():
    if "nc" not in _NC_CACHE:
        _NC_CACHE["nc"] = _build_bass()
    return _NC_CACHE["nc"]


def make_in_maps(x: np.ndarray, W: np.ndarray, b: np.ndarray):
    import ml_dtypes

    bf = ml_dtypes.bfloat16
    scale = np.float32(1.0 / np.sqrt(HEAD_DIM))

    # tril-style mask: keep score[key p, query j] when j >= p; cols >= 128
    # (beyond the diagonal tile) are always kept.
    trim = np.ones((P, 512), dtype=bf)
    trim[:, 0:128] = (np.arange(128)[None, :] >= np.arange(P)[:, None]).astype(bf)
    trim = np.ascontiguousarray(trim)

    xts = [np.ascontiguousarray(x[bb].T.astype(bf)) for bb in range(B)]
    in_maps = []
    for core in range(NCORES):
        bb, g = core // 4, core % 4
        o0 = g * 256
        wq = W[o0 : o0 + 256, :] * scale
        wk = W[C + o0 : C + o0 + 256, :]
        wv = W[2 * C + o0 : 2 * C + o0 + 256, :]
        wt = np.ascontiguousarray(
            np.concatenate([wq.T, wk.T, wv.T], axis=1).astype(bf)
        )
        bq = b[o0 : o0 + 256] * scale
        bk = b[C + o0 : C + o0 + 256]
        bvv = np.ascontiguousarray(b[2 * C + o0 : 2 * C + o0 + 256].astype(bf))
        bcol = np.ascontiguousarray(np.concatenate([bq, bk]), dtype=np.float32)
        in_maps.append(
            {"xt": xts[bb], "wt": wt, "bcol": bcol, "bv": bvv, "trim": trim}
        )
    return in_maps


def kernel(x: np.ndarray, W: np.ndarray, b: np.ndarray) -> np.ndarray:
    from concourse.bass_utils import run_bass_kernel_spmd

    x = np.asarray(x, dtype=np.float32)
    W = np.asarray(W, dtype=np.float32)
    b = np.asarray(b, dtype=np.float32)

    nc = _get_nc()
    in_maps = make_in_maps(x, W, b)
    res = run_bass_kernel_spmd(nc, in_maps, core_ids=list(range(NCORES)))

    y = np.empty((B, T, C), dtype=np.float32)
    for core in range(NCORES):
        bb, g = core // 4, core % 4
        y[bb, :, g * 256 : (g + 1) * 256] = res.results[core]["out"].T
    return y


# revision 27
# speedup vs baseline: 1.0068x; 1.0061x over previous
"""Causal ReLU-attention (no softmax) fused kernel for TRN2, 8 NeuronCores.

Reference computation (B=2, T=2048, C=1024, H=16, D=64):
    qkv = x @ W.T + b ; q,k,v split; per (b,h): y = relu(tril(q k^T / sqrt(D))) @ v

Sharding: core c handles batch b = c//4 and heads 4*(c%4) .. 4*(c%4)+3.
Each core is fully independent (no collectives).

Design (all matmuls bf16, fp32 PSUM accumulate; ~117us vs 155-166us
fp32r baseline):
  - Inputs host-converted to bf16: halves input DMA (5.9MB/core) and
    removes the fp32r <256-col matmul penalty.
  - Phase 1 (projection): per T-half, q/k matmuls interleaved per
    contraction chunk (tracks DMA arrival), 8 single-bank PSUM tiles;
    v bias folded into the accumulation as a rank-1 (ones x bv) matmul
    so every evac is a plain ACT/DVE copy writing bf16.  Input DMAs
    split across both HW queues (SP + ACT) in consumption order.
  - Phase 2 (attention): flat SW pipeline over all (qc, hp) tiles
    (order qc=1,2,0,3 so evac-heavy qc0 sits between PE-heavy windows;
    diag blocks spread between off-diag ones).  KEY perf fact: two
    matmuls with disjoint 64-row PE row-groups (the two heads' STs,
    64-dim contraction) OR disjoint 64-col col-groups (the two heads'
    AVs into one [128,512] yps bank) DUAL-ISSUE on the PE -- phase-2
    runs both heads' ST and AV pairs concurrently.  Do NOT interleave
    single ST/AV instructions; emit pairs back-to-back.
  - ReLU evacs rotate ACT/DVE (the only PSUM-capable engines; GPSIMD
    cannot touch PSUM).  Diag blocks alternate between a DVE fused
    scalar_tensor_tensor (relu + tril-mask multiply) and ACT relu +
    Pool in-SBUF mask multiply.  Phase 2 is evac-throughput-bound.
  - yps [128,512] PSUM holds both heads of a pair -> one copy + one
    contiguous output DMA per (hp,qc); double-buffered across its.
"""

import numpy as np

N_EMBD = 1024
N_HEAD = 16
HEAD_DIM = 64
B, T, C = 2, 2048, N_EMBD
NCORES = 8
P = 128
KC = C // P  # 8 contraction chunks
HT = T // 2  # 1024, half of T
NQC = T // 512  # 4 query chunks
NKB = T // P  # 16 key blocks

_NC_CACHE = {}


def _build_bass():
    import concourse.bass as bass
    from concourse import bacc, mybir
    from concourse.tile import TileContext

    f32 = mybir.dt.float32
    bf16 = mybir.dt.bfloat16

    nc = bacc.Bacc(None, target_bir_lowering=False)
    xt = nc.declare_dram_parameter("xt", [C, T], bf16, isOutput=False)
    wt = nc.declare_dram_parameter("wt", [C, 768], bf16, isOutput=False)
    bcol = nc.declare_dram_parameter("bcol", [512], f32, isOutput=False)
    bv = nc.declare_dram_parameter("bv", [256], bf16, isOutput=False)
    trim = nc.declare_dram_parameter("trim", [P, 512], bf16, isOutput=False)
    out = nc.declare_dram_parameter("out", [256, T], f32, isOutput=True)

    xt_r = xt[:, :].rearrange("(c p) t -> c p t", p=P)
    wt_r = wt[:, :].rearrange("(c p) o -> c p o", p=P)

    with TileContext(nc) as tc:
        with (
            tc.tile_pool(name="const", bufs=1) as const_pool,
            tc.tile_pool(name="qkv", bufs=1) as qkv_pool,
        ):
            # ---- input tiles (per-chunk so deps are fine-grained) ----
            wt_sb = [
                const_pool.tile([P, 768], bf16, name=f"wt{c}") for c in range(KC)
            ]
            xt_sb = [
                [
                    const_pool.tile([P, HT], bf16, name=f"xt{h}_{c}")
                    for c in range(KC)
                ]
                for h in range(2)
            ]
            bqk_sb = const_pool.tile([P, 4], f32)
            bv_sb = const_pool.tile([1, 256], bf16)
            ones_sb = const_pool.tile([1, P], bf16)
            trim_sb = const_pool.tile([P, 512], bf16)

            nc.vector.memset(ones_sb, 1.0)

            # DMA issue in consumption order, split across both HW queues:
            # wt chunks on the ACT queue, xt chunks on the SP queue, so the
            # first matmul's inputs stream concurrently.
            for c in range(KC):
                nc.scalar.dma_start(out=wt_sb[c], in_=wt_r[c])
                eng = nc.sync if c % 2 == 0 else nc.scalar
                eng.dma_start(out=xt_sb[0][c], in_=xt_r[c][:, 0:HT])
                if c == 1:
                    nc.sync.dma_start(
                        out=bqk_sb, in_=bcol[:].rearrange("(m p) -> p m", p=P)
                    )
                    nc.sync.dma_start(
                        out=bv_sb, in_=bv[:].rearrange("(a b) -> a b", a=1)
                    )
                    nc.sync.dma_start(out=trim_sb, in_=trim[:, :])
            for c in range(KC):
                eng = nc.sync if c % 2 == 0 else nc.scalar
                eng.dma_start(out=xt_sb[1][c], in_=xt_r[c][:, HT:T])

            q_sb = qkv_pool.tile([P, 2, T], bf16)
            k_sb = qkv_pool.tile([P, 2, T], bf16)
            v_sb = qkv_pool.tile([P, NKB, 256], bf16)

            # ---- phase 1: projection, by T-halves ----
            with tc.tile_pool(name="psum1", bufs=1, space="PSUM") as psum1:
                for half in range(2):
                    t0c = half * HT
                    pq = {}
                    pk = {}
                    for m in range(2):
                        for n in range(2):
                            pq[(m, n)] = psum1.tile(
                                [P, 512], f32, tag=f"p{2 * m + n}",
                                name=f"pq{half}{m}{n}",
                            )
                            pk[(m, n)] = psum1.tile(
                                [P, 512], f32, tag=f"p{4 + 2 * m + n}",
                                name=f"pk{half}{m}{n}",
                            )
                    for c in range(KC):
                        for m in range(2):
                            for n in range(2):
                                nc.tensor.matmul(
                                    pq[(m, n)],
                                    wt_sb[c][:, m * P : (m + 1) * P],
                                    xt_sb[half][c][:, n * 512 : (n + 1) * 512],
                                    start=(c == 0),
                                    stop=(c == KC - 1),
                                )
                        for m in range(2):
                            for n in range(2):
                                nc.tensor.matmul(
                                    pk[(m, n)],
                                    wt_sb[c][:, 256 + m * P : 256 + (m + 1) * P],
                                    xt_sb[half][c][:, n * 512 : (n + 1) * 512],
                                    start=(c == 0),
                                    stop=(c == KC - 1),
                                )
                    # evacs: q on ACT (bias via activation), k on DVE
                    for m in range(2):
                        for n in range(2):
                            sl = slice(t0c + n * 512, t0c + (n + 1) * 512)
                            nc.scalar.activation(
                                out=q_sb[:, m, sl],
                                in_=pq[(m, n)],
                                func=mybir.ActivationFunctionType.Identity,
                                bias=bqk_sb[:, m : m + 1],
                                scale=1.0,
                            )
                            nc.vector.tensor_scalar_add(
                                k_sb[:, m, sl], pk[(m, n)], bqk_sb[:, 2 + m : 3 + m]
                            )
                    # v: 4 psum tiles of [128,512], each covering 2 t-blocks;
                    # reuse q's tags (q evacs are issued first).  bv is folded
                    # in as a rank-1 (ones x bv) matmul so the evac is a copy.
                    for vt in range(4):
                        pv = psum1.tile(
                            [P, 512], f32, tag=f"p{vt}", name=f"pv{half}{vt}"
                        )
                        for tl in range(2):
                            tb8 = vt * 2 + tl
                            for c in range(KC):
                                nc.tensor.matmul(
                                    pv[:, tl * 256 : (tl + 1) * 256],
                                    xt_sb[half][c][:, tb8 * P : (tb8 + 1) * P],
                                    wt_sb[c][:, 512:768],
                                    start=(c == 0),
                                    stop=False,
                                )
                            nc.tensor.matmul(
                                pv[:, tl * 256 : (tl + 1) * 256],
                                ones_sb[0:1, :],
                                bv_sb[0:1, :],
                                start=False,
                                stop=True,
                            )
                        vdst = v_sb[:, half * 8 + vt * 2 : half * 8 + vt * 2 + 2, :]
                        pvr = pv.rearrange("p (a b) -> p a b", a=2)
                        if vt % 2 == 0:
                            nc.scalar.copy(vdst, pvr)
                        else:
                            nc.vector.tensor_copy(vdst, pvr)

            # ---- phase 2: attention, flat pipeline over (qc, hp) ----
            with (
                tc.tile_pool(name="stsb2", bufs=1) as stsb_pool,
                tc.tile_pool(name="ysb2", bufs=1) as ysb_pool,
                tc.tile_pool(name="pst", bufs=1, space="PSUM") as pst_pool,
                tc.tile_pool(name="py", bufs=1, space="PSUM") as py_pool,
            ):
                # iteration list: evac-heavy qc0 windows sandwiched between
                # PE-heavy qc2/qc3 windows; diag blocks spread between
                # off-diag ones so ST bursts never exceed the stps slots.
                its = []
                for qc in (1, 2, 0, 3):
                    for hp in range(2):
                        off = list(range(4 * qc))
                        diag = [4 * qc + dd for dd in range(4)]
                        kbs = []
                        for i4 in range(4):
                            kbs += off[i4 * qc : (i4 + 1) * qc] + [diag[i4]]
                        its.append((hp, qc, kbs))

                yps = {}  # it_idx -> psum tile (both heads, 1 bank)
                n_av = {}  # it_idx -> AV blocks emitted
                eload = [0.0, 0.0]  # ACT, DVE cumulative evac cost

                def emit_st(it_idx, hp, qc, kb):
                    d = kb - 4 * qc
                    c0 = P * d if d > 0 else 0
                    stps = pst_pool.tile(
                        [P, 2, 512], f32, tag="stps", bufs=3, name="stps"
                    )
                    stsb = stsb_pool.tile(
                        [P, 2, 512], bf16, tag="stsb", bufs=8, name="stsb"
                    )
                    for hh in range(2):
                        off = hh * 64
                        nc.tensor.matmul(
                            stps[:, hh, c0:512],
                            k_sb[off : off + 64, hp, kb * P : (kb + 1) * P],
                            q_sb[off : off + 64, hp, qc * 512 + c0 : (qc + 1) * 512],
                            start=True,
                            stop=True,
                        )
                    L = 512 - c0
                    act_light = eload[0] * 1.25 <= eload[1]
                    if d >= 0 and not act_light:
                        # diag: fused relu + causal-triangle mask on DVE
                        mask = bass.AP(
                            tensor=trim_sb.tensor,
                            offset=trim_sb.offset,
                            ap=[trim_sb.ap[0], [0, 2], [1, L]],
                        )
                        nc.vector.scalar_tensor_tensor(
                            out=stsb[:, :, c0:512],
                            in0=stps[:, :, c0:512],
                            scalar=0.0,
                            in1=mask,
                            op0=mybir.AluOpType.max,
                            op1=mybir.AluOpType.mult,
                        )
                        eload[1] += 2 * L * 1.04 + 170
                    else:
                        # plain relu on the less-loaded engine
                        if act_light:
                            nc.scalar.activation(
                                out=stsb[:, :, c0:512],
                                in_=stps[:, :, c0:512],
                                func=mybir.ActivationFunctionType.Relu,
                            )
                            eload[0] += 2 * L * 0.83 + 200
                        else:
                            nc.vector.tensor_scalar_max(
                                stsb[:, :, c0:512], stps[:, :, c0:512], 0.0
                            )
                            eload[1] += 2 * L * 1.04 + 170
                        if d >= 0:
                            # causal triangle applied in SBUF on the Pool
                            # engine (per-head 2D ops; Pool cannot read PSUM)
                            for hh in range(2):
                                nc.gpsimd.tensor_tensor(
                                    out=stsb[:, hh, c0 : c0 + P],
                                    in0=stsb[:, hh, c0 : c0 + P],
                                    in1=trim_sb[:, 0:P],
                                    op=mybir.AluOpType.mult,
                                )
                    return stsb

                def emit_av(it_idx, hp, qc, kb, stsb, nblocks):
                    d = kb - 4 * qc
                    c0 = P * d if d > 0 else 0
                    if it_idx not in yps:
                        yps[it_idx] = py_pool.tile(
                            [P, 512], f32, tag=f"yps{it_idx % 2}", name=f"yps{it_idx}"
                        )
                        n_av[it_idx] = 0
                    i = n_av[it_idx]
                    n_av[it_idx] = i + 1
                    for hh in range(2):
                        h = 2 * hp + hh
                        nc.tensor.matmul(
                            yps[it_idx][hh * 64 : (hh + 1) * 64, c0:512],
                            v_sb[:, kb, h * 64 : (h + 1) * 64],
                            stsb[:, hh, c0:512],
                            start=(i == 0),
                            stop=(i == nblocks - 1),
                        )
                    if i == nblocks - 1:
                        # drain yps -> sbuf -> dram (both heads at once)
                        ysb = ysb_pool.tile(
                            [P, 512], f32, tag="ysb", bufs=2, name="ysb"
                        )
                        if eload[0] * 1.25 <= eload[1]:
                            nc.scalar.copy(ysb, yps[it_idx])
                            eload[0] += 512 * 0.83 + 200
                        else:
                            nc.vector.tensor_copy(ysb, yps[it_idx])
                            eload[1] += 512 * 1.04 + 170
                        nc.sync.dma_start(
                            out=out[hp * P : (hp + 1) * P, qc * 512 : (qc + 1) * 512],
                            in_=ysb,
                        )

                SKEW = 4  # in blocks
                pending = []
                for it_idx, (hp, qc, kbs) in enumerate(its):
                    nblocks = len(kbs)
                    for kb in kbs:
                        stsb = emit_st(it_idx, hp, qc, kb)
                        pending.append((it_idx, hp, qc, kb, stsb, nblocks))
                        if len(pending) > SKEW:
                            emit_av(*pending.pop(0))
                for args in pending:
                    emit_av(*args)

    nc.compile()
    return nc


def _get_nc():
    if "nc" not in _NC_CACHE:
        _NC_CACHE["nc"] = _build_bass()
    return _NC_CACHE["nc"]


def make_in_maps(x: np.ndarray, W: np.ndarray, b: np.ndarray):
    import ml_dtypes

    bf = ml_dtypes.bfloat16
    scale = np.float32(1.0 / np.sqrt(HEAD_DIM))

    # tril-style mask: keep score[key p, query j] when j >= p; cols >= 128
    # (beyond the diagonal tile) are always kept.
    trim = np.ones((P, 512), dtype=bf)
    trim[:, 0:128] = (np.arange(128)[None, :] >= np.arange(P)[:, None]).astype(bf)
    trim = np.ascontiguousarray(trim)

    xts = [np.ascontiguousarray(x[bb].T.astype(bf)) for bb in range(B)]
    in_maps = []
    for core in range(NCORES):
        bb, g = core // 4, core % 4
        o0 = g * 256
        wq = W[o0 : o0 + 256, :] * scale
        wk = W[C + o0 : C + o0 + 256, :]
        wv = W[2 * C + o0 : 2 * C + o0 + 256, :]
        wt = np.ascontiguousarray(
            np.concatenate([wq.T, wk.T, wv.T], axis=1).astype(bf)
        )
        bq = b[o0 : o0 + 256] * scale
        bk = b[C + o0 : C + o0 + 256]
        bvv = np.ascontiguousarray(b[2 * C + o0 : 2 * C + o0 + 256].astype(bf))
        bcol = np.ascontiguousarray(np.concatenate([bq, bk]), dtype=np.float32)
        in_maps.append(
            {"xt": xts[bb], "wt": wt, "bcol": bcol, "bv": bvv, "trim": trim}
        )
    return in_maps


def kernel(x: np.ndarray, W: np.ndarray, b: np.ndarray) -> np.ndarray:
    from concourse.bass_utils import run_bass_kernel_spmd

    x = np.asarray(x, dtype=np.float32)
    W = np.asarray(W, dtype=np.float32)
    b = np.asarray(b, dtype=np.float32)

    nc = _get_nc()
    in_maps = make_in_maps(x, W, b)
    res = run_bass_kernel_spmd(nc, in_maps, core_ids=list(range(NCORES)))

    y = np.empty((B, T, C), dtype=np.float32)
    for core in range(NCORES):
        bb, g = core // 4, core % 4
        y[bb, :, g * 256 : (g + 1) * 256] = res.results[core]["out"].T
    return y
